# revision 40
# baseline (speedup 1.0000x reference)
"""Conv2d 3x3 (pad 1, stride 1) + bias on 8 Trainium2 cores.

Problem: x [32,128,56,56] f32, weights [256,128,3,3] f32, bias [256] f32
         -> out [32,256,56,56] f32.

Strategy
--------
Data-parallel over batch (4 images/core) + 1D Winograd F(2,3) along W.

For each output pair (2u, 2u+1) and each vertical tap kh, the 3-tap
horizontal conv costs 4 multiplies instead of 6: with d0..d3 the 4
padded inputs around the pair,
  t0 = d0-d2, t1 = d1+d2, t2 = d2-d1, t3 = d1-d3
  m_p = sum_cin sum_kh w'_p[kh] * t_p[row r+kh]
        (w'_0=g0, w'_1=(g0+g1+g2)/2, w'_2=(g0-g1+g2)/2, w'_3=g2)
  out[2u]   = m0+m1+m2+bias
  out[2u+1] = m1-m2-m3+bias
PE work drops from 9 to 6 matmul-columns per output pixel (and the
junk 57-stride column of a direct kernel disappears): 150.5K cols/core
= 62.7us at 2.4GHz vs 95.8us direct.

Layout: the host splits padded rows into even/odd column planes stored
row-major ([58 rows][2 planes][29 cols]) so row-chunk DMAs write
contiguous ranges (the tile dep tracker uses bounding intervals; an
interleaved layout creates false chunk->transform deps). The input
stays compact (861KB/image): a host-pre-shifted 4-plane layout (2x the
bytes) measurably raised DMA power -> chip throttle -> PE clock loss
worth more than its DVE savings. The t-plane transforms run as
HALF-plane ops (<=812 cols keeps the DVE 2x mode; whole-plane strided
ops measured 2-4x slower on HW). The t-planes [cin, 58*28] use flat
row-stride 28: vertical tap kh of a group at flat col lo is the
constant offset lo + kh*28, so matmuls run seamlessly across rows.

Per group of N=392 pair-cols: 12 matmuls (4 m-banks x 3 kh) accumulate
into 4 single-bank PSUM tiles; two groups double-buffer across the 8
banks (784-col double-groups measured slower: single-buffered PSUM
couples the pipeline and lengthens the tail's serial drain chain).
GpSimd cannot read PSUM and two-tensor DVE ops may read at most one
PSUM operand, so the A^T combine is:
  ACT:    a1 = Ident(m1+bias), a2 = Copy(m2), a3 = Copy(-m3)
  DVE:    w1 = a1-a2 (sbuf 2x), u0 = m0+a1 (the one psum op), out0 = u0+a2
  GpSimd: out1 = w1+a3 (sbuf only)
Outputs stay as separate even/odd bf16 planes (the host interleaves
and widens to f32; tolerance is 2e-2, bf16 out lands ~7e-3), one DMA
per group.

Startup: warmup matmuls ramp the PE clock while the first DMA wave
lands (chunk 0, bias, weights in first-use order, chunk 1); chunks 2-4
are gated behind the last warmup matmul so they can't steal first-wave
DMA bandwidth (the Tile scheduler hoists dep-free DMAs past queue
order, so position alone cannot hold them back). Image 0 half 0 is
start-tapered (112/280 cols) so the first matmuls need only 7 input
rows. Image b+1 prefetches as one DMA gated on image b's first a1; its
t-planes build as half-plane ops spread one-per-slot through image b
(plane 3 in image b+1's own slot 0, emitted pre-matmul - dep tracking
is program-order based). The last half tapers (392,392,392,280,112),
its neighbours' output DMAs shift to the ACT queue, and the final
group's drain chain avoids the possibly-backlogged GpSimd queue and
ships its two output planes on separate queues.
"""

import numpy as np
import ml_dtypes

import concourse.bacc as bacc
import concourse.mybir as mybir
import concourse.tile as tile
from concourse.bass_utils import run_bass_kernel_spmd

B, CIN, H, W = 32, 128, 56, 56
COUT = 256
NCORES = 8
BLOC = B // NCORES  # images per core
NR = H + 2  # 58 padded rows
PW = W // 2 + 1  # 29 even/odd plane cols
PC = W // 2  # 28 output pairs per row
PLANE = NR * PC  # 1624 flat t-plane cols
NPAIR = H * PC  # 1568 output pair-cols per image-half
NWARM = 4

# Weight stationary order per half = first-use order: m1, m2, m0, m3.
WORDER = [1, 2, 0, 3]
PIDX = {p: i for i, p in enumerate(WORDER)}

NORM_GROUPS = [(0, 392), (392, 392), (784, 392), (1176, 392)]
# Image 0 half 0: start-taper so the first matmuls need only 7 rows.
FIRST_GROUPS = [(0, 112), (112, 280), (392, 392), (784, 392), (1176, 392)]
# Last half: end-taper to shorten the final drain + DMA chain.
LAST_GROUPS = [(0, 392), (392, 392), (784, 392), (1176, 280), (1456, 112)]
# Image 0 xe/xo row chunks (DMA + transform granularity). Group g_i of
# FIRST_GROUPS needs plane rows < 6/16/30/44/58.
CHUNKS0 = [(0, 7), (7, 17), (17, 31), (31, 45), (45, 58)]

_nc_cache = None


def _build():
    f32 = mybir.dt.float32
    bf16 = mybir.dt.bfloat16
    COPY = mybir.ActivationFunctionType.Copy
    IDENT = mybir.ActivationFunctionType.Identity
    nc = bacc.Bacc("TRN2", target_bir_lowering=False)
    x_d = nc.dram_tensor("xeo", [BLOC, CIN, NR, 2, PW], bf16, kind="ExternalInput")
    w_d = nc.dram_tensor("wT", [CIN, 2 * 4 * 3 * 128], bf16, kind="ExternalInput")
    b_d = nc.dram_tensor("bias2", [128, 2], f32, kind="ExternalInput")
    o_d = nc.dram_tensor("out", [BLOC, 2, 128, 2, NPAIR], bf16, kind="ExternalOutput")

    def wcol(h, p, kh):
        return ((h * 4 + PIDX[p]) * 3 + kh) * 128

    with tile.TileContext(nc) as tc:
        with (
            tc.tile_pool(name="wpool", bufs=1) as wpool,
            tc.tile_pool(name="xpool", bufs=2) as xpool,
            tc.tile_pool(name="tpool", bufs=8) as tpool,
            tc.tile_pool(name="upool", bufs=3) as upool,
            tc.tile_pool(name="vpool", bufs=3) as vpool,
            tc.tile_pool(name="opool", bufs=4) as opool,
            tc.tile_pool(name="psum", bufs=8, space="PSUM") as psum,
        ):
            wsb = wpool.tile([CIN, 2 * 4 * 3 * 128], bf16)
            bsb = wpool.tile([128, 2], f32)
            wub = wpool.tile([128, 512], bf16)
            dmy = wpool.tile([128, 2], bf16)
            nc.vector.memset(wub[:], 0.0)
            # Dummy Identity activation: pulls the ~1.3us activation
            # table load to the front of the ACT queue (its engine
            # queue depth is 0, so a late table load would stall it).
            nc.scalar.activation(dmy[:], wub[:, :2], IDENT)

            xeos = [xpool.tile([CIN, NR, 2, PW], bf16, tag="xeo", name="xeo0")]
            tpls = [
                [
                    tpool.tile([CIN, PLANE], bf16, tag="tp", name=f"tp0_{p}")
                    for p in range(4)
                ]
            ]

            # PE warmup: matmul 1 issues as soon as the memset lands;
            # 2-4 keep the clock ramping while the first wave lands.
            wup = psum.tile([128, 512], f32, tag="pt", name="wup")
            for _ in range(NWARM):
                nc.tensor.matmul(
                    wup[:], lhsT=wub[:, :128], rhs=wub[:], start=True, stop=True
                )
            xeo0 = xeos[0]
            # WAW touches: keep chunks 2-4 out of the critical first
            # DMA wave (the Tile scheduler hoists dep-free DMAs past
            # queue order). On GpSimd via the dummy activation's SBUF
            # output (GpSimd cannot read the warmup PSUM tile), so no
            # critical queue ever waits on the gate chain.
            for (r0, r1) in CHUNKS0[2:]:
                nc.gpsimd.tensor_scalar_mul(
                    xeo0[:, r0, 0, 0:2], dmy[:, 0:2], 0.0
                )

            # Startup DMA wave, ordered by first-use deadline.
            nc.sync.dma_start(xeo0[:, 0:7], x_d[0, :, 0:7])
            nc.scalar.dma_start(bsb[:], b_d[:])
            nc.scalar.dma_start(wsb[:, 0:384], w_d[:, 0:384])
            nc.scalar.dma_start(wsb[:, 384:1536], w_d[:, 384:1536])
            nc.scalar.dma_start(xeo0[:, 7:17], x_d[0, :, 7:17])
            nc.scalar.dma_start(wsb[:, 1536:2304], w_d[:, 1536:2304])
            nc.scalar.dma_start(wsb[:, 2304:3072], w_d[:, 2304:3072])
            for (r0, r1) in CHUNKS0[2:]:
                nc.sync.dma_start(xeo0[:, r0:r1], x_d[0, :, r0:r1])

            def transform(bi, r0, r1, only=None):
                """t-plane rows [r0,r1) for image slot bi (DVE).

                Strided reads (28-of-29 cols per row); keep r1-r0 <= 29
                so the op stays inside the DVE 2x-mode size window.
                """
                xeo = xeos[bi]
                tp = tpls[bi]
                xe = lambda a, b_: xeo[:, r0:r1, 0, a:b_]
                xo = lambda a, b_: xeo[:, r0:r1, 1, a:b_]
                ops = {
                    0: (nc.vector.tensor_sub, xe(0, PC), xe(1, PC + 1)),
                    1: (nc.vector.tensor_add, xo(0, PC), xe(1, PC + 1)),
                    2: (nc.vector.tensor_sub, xe(1, PC + 1), xo(0, PC)),
                    3: (nc.vector.tensor_sub, xo(0, PC), xo(1, PC + 1)),
                }
                order = [only] if only is not None else WORDER
                for p in order:
                    fn, a, b_ = ops[p]
                    t3d = tp[p][:].rearrange("c (r u) -> c r u", r=NR)
                    fn(t3d[:, r0:r1, :], a, b_)

            transform(0, *CHUNKS0[0])
            transform(0, *CHUNKS0[1])

            def do_group(b, h, lo, n, last_group=False, pre_drains=None,
                         pre_mm=None, prefetch=False):
                if pre_mm is not None:
                    pre_mm()
                tp = tpls[b]
                pts = {}
                for p in WORDER:
                    pts[p] = psum.tile(
                        [128, 392], f32, tag="pt", name=f"pt_b{b}h{h}l{lo}p{p}"
                    )
                    for kh in range(3):
                        c = wcol(h, p, kh)
                        nc.tensor.matmul(
                            pts[p][:, :n],
                            lhsT=wsb[:, c : c + 128],
                            rhs=tp[p][:, lo + kh * PC : lo + kh * PC + n],
                            start=(kh == 0),
                            stop=(kh == 2),
                        )
                a1 = vpool.tile([128, 392], bf16, tag="a1")
                a2 = vpool.tile([128, 392], bf16, tag="a2")
                a3 = vpool.tile([128, 392], bf16, tag="a3")
                u0 = upool.tile([128, 392], bf16, tag="u0")
                w1 = upool.tile([128, 392], bf16, tag="w1")
                ot = opool.tile([128, 2, 392], bf16, tag="ot")
                bvec = bsb[:, h : h + 1]
                nc.scalar.activation(a1[:, :n], pts[1][:, :n], IDENT, bias=bvec)
                nc.scalar.activation(a2[:, :n], pts[2][:, :n], COPY)
                nc.scalar.activation(a3[:, :n], pts[3][:, :n], COPY, scale=-1.0)
                if prefetch:
                    # Prefetch next image's xe/xo, gated behind this
                    # group's a1 (an early 861KB prefetch would starve
                    # the transfers gating the PE).
                    xqn = xeos[b + 1]
                    nc.gpsimd.tensor_scalar_mul(
                        xqn[:, 0, 0, 0:2], a1[:, 0:2], 0.0
                    )
                    nc.sync.dma_start(xqn[:], x_d[b + 1])
                # out0 = (m0 + a1) + a2 ; out1 = (a1 - a2) + a3
                # u0 first: it releases the m0 PSUM bank, the longest
                # pole for the bank ring; a late u0 stalls the PE and
                # re-throttles the clock ramp.
                nc.vector.tensor_add(u0[:, :n], pts[0][:, :n], a1[:, :n])
                nc.vector.tensor_sub(w1[:, :n], a1[:, :n], a2[:, :n])
                nc.vector.tensor_add(ot[:, 0, :n], u0[:, :n], a2[:, :n])
                out1_eng = nc.vector if last_group else nc.gpsimd
                out1_eng.tensor_add(ot[:, 1, :n], w1[:, :n], a3[:, :n])
                if pre_drains is not None:
                    # Transform jobs go AFTER the drains on the DVE
                    # queue: their deadlines have slots of slack, the
                    # drains' bank releases do not.
                    pre_drains()
                if last_group:
                    nc.sync.dma_start(o_d[b, h, :, 0, lo : lo + n], ot[:, 0, :n])
                    nc.scalar.dma_start(o_d[b, h, :, 1, lo : lo + n], ot[:, 1, :n])
                else:
                    # Output DMAs near the kernel end ride the ACT
                    # queue so the SP ring is clear for the last group.
                    oq = (
                        nc.scalar
                        if (b == BLOC - 1 and h == 1 and lo >= 784)
                        else nc.sync
                    )
                    oq.dma_start(o_d[b, h, :, :, lo : lo + n], ot[:, :, :n])

            # Transform jobs per (image, slot): slot = h*4 + gi for
            # normal images (image 0 half 0 has 5 groups -> 9 slots).
            # Image 0's early slots carry its chunks 2-4; halves of
            # planes 1/2/0 for image b+1 build in image b's late slots
            # (after its prefetch lands); plane 3 builds in image
            # b+1's own slot 0, before its matmuls.
            def half(bi, p, hi):
                r = (0, 29) if hi == 0 else (29, NR)
                return lambda bi=bi, p=p, r=r: transform(bi, r[0], r[1], only=p)

            def chunk(ci):
                return lambda ci=ci: transform(0, *CHUNKS0[ci])

            jobs = {b: {} for b in range(BLOC)}
            jobs[0][1] = [chunk(2)]
            jobs[0][2] = [chunk(3)]
            jobs[0][3] = [chunk(4)]
            for b in range(BLOC):
                nslots = 9 if b in (0, BLOC - 1) else 8
                if b + 1 < BLOC:
                    s = nslots - 5
                    jobs[b].setdefault(s, []).append(half(b + 1, 1, 0))
                    jobs[b].setdefault(s + 1, []).append(half(b + 1, 1, 1))
                    jobs[b].setdefault(s + 2, []).append(half(b + 1, 2, 0))
                    jobs[b].setdefault(s + 3, []).append(half(b + 1, 2, 1))
                    jobs[b].setdefault(s + 4, []).extend(
                        [half(b + 1, 0, 0), half(b + 1, 0, 1)]
                    )
                    jobs[b + 1].setdefault(0, []).extend(
                        [half(b + 1, 3, 0), half(b + 1, 3, 1)]
                    )

            for b in range(BLOC):
                if b + 1 < BLOC:
                    xeos.append(
                        xpool.tile([CIN, NR, 2, PW], bf16, tag="xeo",
                                   name=f"xeo{b+1}")
                    )
                    tpls.append(
                        [
                            tpool.tile([CIN, PLANE], bf16, tag="tp",
                                       name=f"tp{b+1}_{p}")
                            for p in range(4)
                        ]
                    )
                slot = 0
                for h in range(2):
                    if b == 0 and h == 0:
                        groups = FIRST_GROUPS
                    elif b == BLOC - 1 and h == 1:
                        groups = LAST_GROUPS
                    else:
                        groups = NORM_GROUPS
                    for gi, (lo, n) in enumerate(groups):
                        jl = jobs[b].get(slot)
                        pre = (
                            None if not jl
                            else (lambda jl=jl: [f() for f in jl])
                        )
                        do_group(
                            b, h, lo, n,
                            last_group=(
                                b == BLOC - 1
                                and h == 1
                                and gi == len(groups) - 1
                            ),
                            pre_drains=pre if slot != 0 else None,
                            pre_mm=pre if slot == 0 else None,
                            prefetch=(
                                h == 0 and gi == 0 and b + 1 < BLOC
                            ),
                        )
                        slot += 1

    nc.compile()
    return nc


def _get_nc():
    global _nc_cache
    if _nc_cache is None:
        _nc_cache = _build()
    return _nc_cache


def _prep_inputs(x, weights, bias):
    x = np.asarray(x, dtype=np.float32)
    weights = np.asarray(weights, dtype=np.float32)
    bias = np.ascontiguousarray(np.asarray(bias, dtype=np.float32))

    xb = x.astype(ml_dtypes.bfloat16)
    xpad = np.pad(xb, ((0, 0), (0, 0), (1, 1), (1, 1)))  # [B,C,58,58]
    xe = xpad[:, :, :, 0::2]  # [B,C,58,29]
    xo = xpad[:, :, :, 1::2]
    xeo = np.ascontiguousarray(np.stack([xe, xo], axis=3))  # [B,C,58,2,29]

    g = weights.reshape(2, 128, CIN, 3, 3)  # [h, co, cin, kh, kw]
    w0 = g[..., 0]
    w1 = (g[..., 0] + g[..., 1] + g[..., 2]) * 0.5
    w2 = (g[..., 0] - g[..., 1] + g[..., 2]) * 0.5
    w3 = g[..., 2]
    wlist = [w0, w1, w2, w3]
    # stack in WORDER; axes [h, p, co, cin, kh] -> [cin, h, p, kh, co]
    wp = np.stack([wlist[p] for p in WORDER], axis=1)
    wT = np.ascontiguousarray(wp.transpose(3, 0, 1, 4, 2)).reshape(
        CIN, 2 * 4 * 3 * 128
    ).astype(ml_dtypes.bfloat16)
    b2 = np.ascontiguousarray(bias.reshape(2, 128).T)  # b2[p,h] = bias[h*128+p]

    return [
        {
            "xeo": np.ascontiguousarray(xeo[i * BLOC : (i + 1) * BLOC]),
            "wT": wT,
            "bias2": b2,
        }
        for i in range(NCORES)
    ]


def _run(inputs, trace=False):
    in_maps = _prep_inputs(inputs["x"], inputs["weights"], inputs["bias"])
    res = run_bass_kernel_spmd(
        _get_nc(), in_maps, core_ids=list(range(NCORES)), trace=trace
    )
    o = np.concatenate([np.asarray(r["out"]) for r in res.results], axis=0)
    # [B, 2h, 128co, 2pl, 1568] bf16 -> [B, 256, 56, 56] f32
    o = o.astype(np.float32).reshape(B, 2, 128, 2, H, PC)
    o = o.transpose(0, 1, 2, 4, 5, 3).reshape(B, COUT, H, W)
    return np.ascontiguousarray(o), res


def kernel(x, weights, bias):
    out, _ = _run({"x": x, "weights": weights, "bias": bias})
    return out
